# revision 18
# baseline (speedup 1.0000x reference)
"""Trainium2 Bass kernel: topo-batched masked-norm NN forward (gnn_message_passing).

Math per topo batch i (reference.py):
    vals = previous layer activations [W]
    n_in[r]  = sum_c M[r,c]
    mean[r]  = (M @ vals)[r] / n_in[r]
    var[r]   = (M @ vals^2)[r] / n_in[r] - mean[r]^2
    rs[r]    = 1/sqrt(var[r] + EPS)
    affine[r]= gamma*rs*( (WM @ vals)[r] - mean[r]*rowWM[r] ) + beta[r]*rowWM[r]
               + bias[r],   WM = W ⊙ M,  rowWM = WM @ 1
    out = silu(affine*gain)*amp   (last batch: identity instead of silu)

Distribution: rows (output neurons) sharded across 8 cores (512 rows/core);
the 4096-vector of activations is all-gathered between batches.

Layout/precision (validated to rel_err ~5.5e-3 in numpy vs the 2e-2 gate):
  * WM is premasked on host and shipped as an fp8(e4m3) hi/lo pair, scaled
    into e4m3's normal range: whi8 = e4m3(64*WM) (with a smallest-subnorm
    fixup so no masked weight quantizes to 0), wlo8 = e4m3(16*(64*WM-whi8)).
    Same bytes as bf16, but fp8 matmuls run in DoubleRow perf mode: 2
    c-blocks (256-deep contraction) per instruction at 0.5 cycles/row —
    the whole sweep is 48 matmuls at ~107 ns instead of 64 at ~213 ns.
  * The 0/1 mask is re-derived ON DEVICE as (whi8 != 0) on the DVE — exact
    thanks to the fixup — and fed to the stats matmuls as fp8.
  * Activations are an fp8 hi/lo pair (stationary side; cost-free rows).
    The lo@lo affine term is kept (free PSUM row).
  * Descaling is FREE: the PSUM->fold transpose is a matmul against a
    constant selector, whose entries fold 1/64, 1/1024, and the EPS*n_in
    term into the combine.
  * n_in, 1/n_in, rowWM and affine constants precomputed on host:
      P0 = gamma*gain, P1 = (bias + beta*rowWM)*gain, amp, rn = 1/n_in
  * rsqrt: Quake seed + one Newton iteration on DVE.
  * DMA queues: weights on the SP HWDGE queue; collective bounce buffers
    on the otherwise-idle Activation HWDGE queue; gpsimd only collectives.
"""

import numpy as np
import ml_dtypes

import concourse.bass as bass
import concourse.bacc as bacc
import concourse.tile as tile
import concourse.mybir as mybir
from concourse import bass_utils

L, W, NC = 8, 4096, 8
NB = L - 1                # 7 topo batches
RPC = W // NC             # 512 rows per core
CB = W // 128             # 32 contraction blocks of 128
RB = RPC // 128           # 4 row blocks of 128 per core
NQ = 4                    # c-block quarters per batch
QJ = CB // NQ             # 8 c-blocks per quarter
UQ = QJ // 2              # 4 DoubleRow pairs per quarter
EPS = 1e-5
WS = 64.0                 # weight scale into e4m3 normal range
WS2 = 16.0                # residual scale

F8 = mybir.dt.float8e4
F32 = mybir.dt.float32
I32 = mybir.dt.int32
ADD = mybir.AluOpType.add
SUB = mybir.AluOpType.subtract
MUL = mybir.AluOpType.mult
NEQ = mybir.AluOpType.not_equal
RSHIFT = mybir.AluOpType.logical_shift_right
DR = mybir.MatmulPerfMode.DoubleRow

_CACHED = None


def _kernel_body(nc, tc, whi_d, wlo_d, xf_d, pf_d, sel_d, y_d):
    with (
        tc.tile_pool(name="const", bufs=1) as constp,
        tc.tile_pool(name="whip", bufs=8) as whip,
        tc.tile_pool(name="wlop", bufs=8) as wlop,
        tc.tile_pool(name="mp", bufs=6) as mp,
        tc.tile_pool(name="vals", bufs=2) as valsp,
        tc.tile_pool(name="ep", bufs=2) as epp,
        tc.tile_pool(name="psum", bufs=1, space="PSUM") as psump,
        tc.tile_pool(name="dram", bufs=2, space="DRAM") as dramp,
    ):
        # ---- persistent: per-row params, folded [128, NB*5*RB] ----
        # s: 0=rn(1/n_in), 1=rowWM, 2=P0(gamma*gain), 3=P1, 4=amp
        params = constp.tile([128, NB * 5 * RB], F32)
        nc.sync.dma_start(out=params[:], in_=pf_d.ap())

        def pslice(i, s):
            o = (i * 5 + s) * RB
            return params[:, o:o + RB]

        # transpose selector rows: 0:5 = [s1hi, s1lo, sqhi, sqlo, n_in],
        # 32:34 = [t_hh, t_hl], 64:66 = [t_lh, t_ll]; entries carry the fp8
        # descale factors and EPS*n_in
        sel = constp.tile([128, 8], F32, name="sel")
        nc.sync.dma_start(out=sel[0:66, :], in_=sel_d.ap())

        # persistent transpose staging tile; only rows 0:5 / 32:34 / 64:66
        # are rewritten per batch, the rest must stay 0
        sb = constp.tile([128, 512], F32, name="sb")
        nc.vector.memset(sb[:, :], 0.0)

        # ---- persistent: per-batch stationary vectors [128, CB*16] fp8 ----
        # col layout per c-block j: [vhi, vlo, sqhi, sqlo, ones, pad*11] —
        # padded to 16 so the DoubleRow pair stride satisfies the ISA's
        # step%16==0 requirement on the dual-fp8 LDWEIGHTS AP
        vstat = constp.tile([128, CB * 16], F8)
        v5 = vstat[:].rearrange("p (j s) -> p j s", s=16)
        nc.vector.memset(v5[:, :, 4], 1.0)

        prev_cc_out = None
        for i in range(NB):
            # ======== weight streaming (vals-independent) + mask derive ====
            whi_t, wlo_t, m_t = [], [], []
            for q in range(NQ):
                wh = whip.tile([128, QJ * RPC], F8, tag="whi", name="whi")
                wl = wlop.tile([128, QJ * RPC], F8, tag="wlo", name="wlo")
                nc.sync.dma_start(
                    out=wh[:].rearrange("p (a b) -> p a b", b=RPC),
                    in_=whi_d[i][:, q * QJ:(q + 1) * QJ, :],
                )
                nc.sync.dma_start(
                    out=wl[:].rearrange("p (a b) -> p a b", b=RPC),
                    in_=wlo_d[i][:, q * QJ:(q + 1) * QJ, :],
                )
                whi_t.append(wh)
                wlo_t.append(wl)
            # derive m = (whi8 != 0) as fp8 0/1 on DVE. Quarters 0,1 emitted
            # before vstat (run inside the previous AllGather window), 2,3
            # after (run under this batch's sweep).
            for q in range(2):
                mq = mp.tile([128, QJ * RPC], F8, tag="m", name="m")
                nc.vector.tensor_scalar(mq[:], whi_t[q][:], 0.0, None, op0=NEQ)
                m_t.append(mq)

            # ======== vals -> vstat ========================================
            vals = valsp.tile([128, CB], F32, tag="vals", name="vals")
            if i == 0:
                nc.scalar.dma_start(out=vals[:], in_=xf_d.ap())
            else:
                # cc payload is fold-major per rank: element (k, p, rb) = row
                # k*512 + rb*128 + p; c-block j = 4k + rb  =>  [p, (k rb)]
                nc.scalar.dma_start(
                    out=vals[:].rearrange("p (k rb) -> p k rb", rb=RB),
                    in_=prev_cc_out.rearrange("(k p rb) -> p k rb",
                                              p=128, rb=RB),
                )
            tmp_a = epp.tile([128, CB], F32, tag="vtmp_a", name="vtmp_a")
            tmp_sq = epp.tile([128, CB], F32, tag="vtmp_sq", name="vtmp_sq")
            nc.vector.tensor_copy(v5[:, :, 0], vals[:])             # vhi8
            nc.vector.tensor_copy(tmp_a[:], v5[:, :, 0])
            nc.vector.tensor_tensor(v5[:, :, 1], vals[:], tmp_a[:], op=SUB)
            nc.vector.tensor_tensor(tmp_sq[:], vals[:], vals[:], op=MUL)
            nc.vector.tensor_copy(v5[:, :, 2], tmp_sq[:])           # sqhi8
            nc.vector.tensor_copy(tmp_a[:], v5[:, :, 2])
            nc.vector.tensor_tensor(v5[:, :, 3], tmp_sq[:], tmp_a[:], op=SUB)

            for q in range(2, NQ):
                mq = mp.tile([128, QJ * RPC], F8, tag="m", name="m")
                nc.vector.tensor_scalar(mq[:], whi_t[q][:], 0.0, None, op0=NEQ)
                m_t.append(mq)

            # ======== matvec sweep (fp8 DoubleRow: 2 c-blocks / matmul) ====
            # ps_af  rows [hi@vhi, hi@vlo];  ps_af2 rows [lo@vhi, lo@vlo];
            # ps_st rows [m@vhi, m@vlo, m@sqhi, m@sqlo, n_in]
            ps_st = psump.tile([128, 512], F32, tag="ps_st", name="ps_st")
            ps_af = psump.tile([128, 512], F32, tag="ps_af", name="ps_af")
            ps_af2 = psump.tile([128, 512], F32, tag="ps_af2", name="ps_af2")
            for u in range(CB // 2):
                q, uq = divmod(u, UQ)
                off = (2 * uq) * RPC
                vp = vstat[:, u * 32:u * 32 + 32].rearrange(
                    "p (two s) -> p two s", s=16)
                rhs_h = whi_t[q][:, off:off + 2 * RPC].rearrange(
                    "p (two n) -> p two n", two=2)
                rhs_l = wlo_t[q][:, off:off + 2 * RPC].rearrange(
                    "p (two n) -> p two n", two=2)
                rhs_m = m_t[q][:, off:off + 2 * RPC].rearrange(
                    "p (two n) -> p two n", two=2)
                st, sp = (u == 0), (u == CB // 2 - 1)
                nc.tensor.matmul(ps_af[0:2, :], lhsT=vp[:, :, 0:2],
                                 rhs=rhs_h, start=st, stop=sp, perf_mode=DR)
                nc.tensor.matmul(ps_af2[0:2, :], lhsT=vp[:, :, 0:2],
                                 rhs=rhs_l, start=st, stop=sp, perf_mode=DR)
                nc.tensor.matmul(ps_st[0:5, :], lhsT=vp[:, :, 0:5],
                                 rhs=rhs_m, start=st, stop=sp, perf_mode=DR)

            # ======== transpose to fold layout =============================
            # copy PSUM row-groups to SBUF partitions 0:5 / 32:34 / 64:66
            # (engines can only address partition bases 0/32/64/96), then per
            # row-block ONE matmul over rows 0:66 against sel lands [128, 8]
            # in PSUM: cols [s1, s2 + EPS*n_in, t1, 0...] (descale included)
            nc.vector.tensor_copy(sb[0:5, :], ps_st[0:5, :])
            nc.vector.tensor_copy(sb[32:34, :], ps_af[0:2, :])
            nc.vector.tensor_copy(sb[64:66, :], ps_af2[0:2, :])
            ps_t = psump.tile([128, RB * 512], F32, tag="ps_t", name="ps_t")
            for rb in range(RB):
                nc.tensor.matmul(
                    ps_t[:, rb * 512:rb * 512 + 8],
                    lhsT=sb[0:66, rb * 128:(rb + 1) * 128],
                    rhs=sel[0:66, :], start=True, stop=True)
            pt3 = ps_t[:].rearrange("p (rb s) -> p rb s", s=512)

            # ======== epilogue (all [128, RB] f32) =========================
            def T(tag):
                return epp.tile([128, RB], F32, tag=tag, name=tag)

            # pt3 cols: 0=s1, 1=s2+EPS*n_in, 2=t1 (all descaled by sel)
            mean, ex2e, msq, vpe = T("mean"), T("ex2e"), T("msq"), T("vpe")
            nc.vector.tensor_tensor(mean[:], pt3[:, :, 0], pslice(i, 0), op=MUL)
            nc.vector.tensor_tensor(ex2e[:], pt3[:, :, 1], pslice(i, 0), op=MUL)
            nc.vector.tensor_tensor(msq[:], mean[:], mean[:], op=MUL)
            nc.vector.scalar_tensor_tensor(
                vpe[:], msq[:], -1.0, ex2e[:], op0=MUL, op1=ADD)
            # rs = 1/sqrt(vpe): Quake seed + 1 Newton iteration (f32, DVE)
            rs, nra, nrb = T("rs"), T("nra"), T("nrb")
            nc.vector.tensor_scalar(
                rs[:].bitcast(I32), vpe[:].bitcast(I32), 1, None, op0=RSHIFT)
            nc.vector.tensor_scalar(
                rs[:].bitcast(I32), rs[:].bitcast(I32), -1, 0x5F3759DF,
                op0=MUL, op1=ADD)
            nc.vector.tensor_tensor(nra[:], rs[:], rs[:], op=MUL)
            nc.vector.tensor_tensor(nrb[:], nra[:], vpe[:], op=MUL)
            nc.vector.tensor_scalar(nrb[:], nrb[:], -0.5, 1.5, op0=MUL, op1=ADD)
            nc.vector.tensor_tensor(rs[:], rs[:], nrb[:], op=MUL)
            # pre = P0*rs*(t1 - mean*rowWM) + P1
            mw, tm, g1g, pre = T("mw"), T("tm"), T("g1g"), T("pre")
            nc.vector.tensor_tensor(mw[:], mean[:], pslice(i, 1), op=MUL)
            nc.vector.tensor_tensor(tm[:], pt3[:, :, 2], mw[:], op=SUB)
            nc.vector.tensor_tensor(g1g[:], pslice(i, 2), rs[:], op=MUL)
            nc.vector.tensor_tensor(pre[:], g1g[:], tm[:], op=MUL)
            nc.vector.tensor_tensor(pre[:], pre[:], pslice(i, 3), op=ADD)
            outv = T("outv")
            if i < NB - 1:
                sil = T("sil")
                nc.scalar.activation(
                    sil[:], pre[:], mybir.ActivationFunctionType.Silu)
                nc.vector.tensor_tensor(outv[:], sil[:], pslice(i, 4), op=MUL)
            else:
                nc.vector.tensor_tensor(outv[:], pre[:], pslice(i, 4), op=MUL)

            # ======== scatter / all-gather =================================
            # payload is fold-major: cc_in[p*RB + rb] = outv[p, rb]. Bounce
            # DMAs ride the idle Activation HWDGE queue; gpsimd only
            # triggers the collective.
            if i < NB - 1:
                cc_in = dramp.tile([RPC], F32, tag="cci", name="cci")
                cc_out = dramp.tile([W], F32, tag="cco", name="cco")
                nc.scalar.dma_start(
                    out=cc_in[:].rearrange("(p rb) -> p rb", rb=RB), in_=outv[:])
                nc.gpsimd.collective_compute(
                    "AllGather",
                    mybir.AluOpType.bypass,
                    replica_groups=[list(range(NC))],
                    ins=[cc_in[:].opt()],
                    outs=[cc_out[:].opt()],
                )
                prev_cc_out = cc_out
            else:
                nc.sync.dma_start(
                    out=y_d.ap().rearrange("(p rb) -> p rb", rb=RB), in_=outv[:])


def _build_program():
    nc = bacc.Bacc("TRN2", target_bir_lowering=False, debug=False,
                   num_devices=NC)
    whi_d = nc.dram_tensor("whi", [NB, 128, CB, RPC], F8, kind="ExternalInput")
    wlo_d = nc.dram_tensor("wlo", [NB, 128, CB, RPC], F8, kind="ExternalInput")
    xf_d = nc.dram_tensor("xf", [128, CB], F32, kind="ExternalInput")
    pf_d = nc.dram_tensor("pf", [128, NB * 5 * RB], F32, kind="ExternalInput")
    sel_d = nc.dram_tensor("sel", [66, 8], F32, kind="ExternalInput")
    y_d = nc.dram_tensor("y", [RPC], F32, kind="ExternalOutput")
    with tile.TileContext(nc) as tc:
        _kernel_body(nc, tc, whi_d, wlo_d, xf_d, pf_d, sel_d, y_d)
    nc.compile()
    return nc


def _pack_inputs(x, weights, masks, biases, gamma, beta, gain, amplification):
    f8 = ml_dtypes.float8_e4m3
    w32 = np.asarray(weights, np.float32)
    m32 = np.asarray(masks, np.float32)
    mask = m32 != 0
    ws = np.where(mask, w32, np.float32(0.0)) * np.float32(WS)
    whi = ws.astype(f8)
    # fixup: masked weights that quantize to 0 get the smallest subnormal so
    # the on-device (whi8 != 0) mask derivation is exact
    fix = (whi == 0) & mask
    if fix.any():
        whi = np.where(fix, (np.sign(ws) * np.float32(2.0 ** -9)).astype(f8),
                       whi)
    whif = whi.astype(np.float32)
    wlo = ((ws - whif) * np.float32(WS2)).astype(f8)

    # input-independent per-row params
    n_in = m32.sum(axis=2, dtype=np.float32)                 # [NB, W]
    rowWM = (whif / np.float32(WS)
             + wlo.astype(np.float32) / np.float32(WS * WS2)).sum(
                 axis=2, dtype=np.float32)
    rn = (1.0 / n_in).astype(np.float32)
    gamma = np.asarray(gamma, np.float32).reshape(NB, W)
    beta = np.asarray(beta, np.float32).reshape(NB, W)
    biases = np.asarray(biases, np.float32).reshape(NB, W)
    gain = np.asarray(gain, np.float32).reshape(NB, W)
    amp = np.asarray(amplification, np.float32).reshape(NB, W)
    P0 = gamma * gain
    P1 = (biases + beta * rowWM) * gain

    # [NB, W(r), W(c)] -> [NB, p, jj, k, rr]  with r = k*RPC+rr, c = jj*128+p
    def fold(a):
        a = a.reshape(NB, NC, RPC, CB, 128)
        return a.transpose(0, 4, 3, 1, 2)

    whi_f, wlo_f = fold(whi), fold(wlo)

    x32 = np.asarray(x, np.float32)
    xf = np.ascontiguousarray(x32.reshape(CB, 128).T)  # [128, CB]

    # params: [NB, W] -> [NB, NC, RB, 128] (row r = k*RPC + rb*128 + p)
    def fold_param(a):
        return np.ascontiguousarray(a, dtype=np.float32).reshape(NB, NC, RB, 128)

    ps = [fold_param(a) for a in (rn, rowWM, P0, P1, amp)]
    pall = np.stack(ps, axis=1)  # [NB, 5, NC, RB, 128]

    # transpose selector: rows [s1hi, s1lo, sqhi, sqlo, n_in] at 0:5,
    # [t_hh, t_hl] at 32:34, [t_lh, t_ll] at 64:66 ->
    # cols [s1, s2 + EPS*n_in, t1, 0...], descales folded in
    sel = np.zeros((66, 8), np.float32)
    sel[0, 0] = sel[1, 0] = 1.0
    sel[2, 1] = sel[3, 1] = 1.0
    sel[4, 1] = EPS
    sel[32, 2] = sel[33, 2] = 1.0 / WS
    sel[64, 2] = sel[65, 2] = 1.0 / (WS * WS2)

    in_maps = []
    for k in range(NC):
        pf = np.ascontiguousarray(
            pall[:, :, k].transpose(3, 0, 1, 2).reshape(128, NB * 5 * RB))
        in_maps.append({
            "whi": whi_f[:, :, :, k, :],
            "wlo": wlo_f[:, :, :, k, :],
            "xf": xf,
            "pf": pf,
            "sel": sel,
        })
    return in_maps


def _get_program():
    global _CACHED
    if _CACHED is None:
        _CACHED = _build_program()
    return _CACHED


def _run(in_maps, **kw):
    nc = _get_program()
    return bass_utils.run_bass_kernel_spmd(
        nc, in_maps, core_ids=list(range(NC)), **kw)


def _unfold_y(shard):
    # y[p*RB + rb] = out[rb*128 + p]
    return np.ascontiguousarray(
        np.asarray(shard, np.float32).reshape(128, RB).T.reshape(-1))


def kernel(x, weights, masks, biases, gamma, beta, gain, amplification):
    in_maps = _pack_inputs(x, weights, masks, biases, gamma, beta, gain,
                           amplification)
    res = _run(in_maps)
    return np.concatenate([_unfold_y(res.results[k]["y"]) for k in range(NC)])


def run_traced(inputs, **kw):
    """For test.py: same as kernel() but with NTFF profiling enabled."""
    in_maps = _pack_inputs(**inputs)
    res = _run(in_maps, trace=True, **kw)
    y = np.concatenate([_unfold_y(res.results[k]["y"]) for k in range(NC)])
    return y, res
